# revision 5
# baseline (speedup 1.0000x reference)
"""CrossAttention (B=4, Tq=Tk=2048, DIM=1024, H=16, DH=64) on 8 TRN2 cores.

Sharding: core = (batch b = core//2) x (head-group hg = core%2, 8 heads each).
Each core computes q/k/v projections for its head group (tensor-parallel over
heads), coord-RoPE, flash-style attention in [k, q] layout (no transposes:
P^T tiles are directly the lhsT of the P@V matmul), and a partial output
projection. Host sums the two partials per batch and adds bo + bv@Wo^T.

Device layouts (per core):
  xTa/mTa [1152, 2048] f32r : x[b].T / memory[b].T padded; row 1024 = 1.0
                              (bias aug row), rows 1025.. = 0.
  wq/wk   [1152, 512] f32r  : (W[hg rows]/scale).T with bias row at 1024.
  wv      [1024, 520] f32r  : head-blocked cols (65 per head: 64 dims + 1
                              spare col that the kernel memsets to 1.0 to
                              produce softmax denominators in the P@V psum).
  wo      [512, 1024] f32r  : Wo[:, hg cols].T
  qcos/qsin/kcos/ksin [128, 2048] f32 : RoPE tables replicated to 128 rows,
                              sin pre-signed (- for first half of each pair
                              block, + for second half).
Output: partial out [2048, 1024] f32 per core.
"""
import numpy as np
from contextlib import ExitStack

import concourse.bacc as bacc
import concourse.tile as tile
from concourse import mybir
from concourse.bass_utils import run_bass_kernel_spmd

F32 = mybir.dt.float32
F32R = mybir.dt.float32r
BF16 = mybir.dt.bfloat16
EXP = mybir.ActivationFunctionType.Exp

B, T, DIM = 4, 2048, 1024
H, DH, NP = 16, 64, 32
KPAD = 1152           # 9 * 128 (1024 data rows + bias row + zero pad)
SCALE = 1.0 / 8.0     # 1/sqrt(DH)

_AXES = np.arange(NP) % 3
_PERIODS = np.geomspace(0.01, 1.0, NP).astype(np.float32)
_INV_FREQ = (2.0 * np.float32(np.pi) / _PERIODS).astype(np.float32)

_SWAP = ((0, 32), (32, 0), (64, 96), (96, 64))


def _rope_tables(coords):
    """coords [T, 3] -> cos128, sin128 [128, T] f32 (sin pre-signed)."""
    ang = coords[:, _AXES] * _INV_FREQ[None, :]        # [T, 32]
    c = np.cos(ang).T.astype(np.float32)               # [32, T]
    s = np.sin(ang).T.astype(np.float32)
    cos128 = np.tile(c, (4, 1))
    sin64 = np.concatenate([-s, s], axis=0)            # [64, T]
    sin128 = np.tile(sin64, (2, 1))
    return np.ascontiguousarray(cos128), np.ascontiguousarray(sin128)


def _prep_core(core, x, memory, query_coords, memory_coords,
               Wq, bq, Wk, bk, Wv, bv, Wo, bo):
    b, hg = core // 2, core % 2
    rows = slice(hg * 512, (hg + 1) * 512)

    def pad_aug(tT):                                    # [1024, T] -> [1152, T]
        out = np.zeros((KPAD, T), np.float32)
        out[:DIM] = tT
        out[DIM] = 1.0
        return out

    def pad_w(W, bias, scale):                          # -> [1152, 512]
        out = np.zeros((KPAD, 512), np.float32)
        out[:DIM] = (W[rows, :] * scale).T
        out[DIM] = bias[rows] * scale
        return out

    wv = np.zeros((DIM, 520), np.float32)
    for j in range(8):
        wv[:, 65 * j:65 * j + 64] = Wv[hg * 512 + 64 * j: hg * 512 + 64 * (j + 1), :].T

    qcos, qsin = _rope_tables(query_coords[b])
    kcos, ksin = _rope_tables(memory_coords[b])
    return {
        "xTa": pad_aug(x[b].T),
        "mTa": pad_aug(memory[b].T),
        "wq": pad_w(Wq, bq, SCALE),
        "wk": pad_w(Wk, bk, 1.0),
        "wv": wv,
        "wo": np.ascontiguousarray(Wo[:, rows].T),
        "qcos": qcos, "qsin": qsin, "kcos": kcos, "ksin": ksin,
    }


def _rope_evict(nc, pool, psum, cos_sb, sin_sb, ncol, out_ap):
    """out = psum * cos + swap_blocks(psum) * sin, written to out_ap (bf16)."""
    t1 = pool.tile([128, ncol], F32, tag="rope_t1")
    nc.vector.tensor_mul(t1[:], psum[:], cos_sb[:])
    t2 = pool.tile([128, ncol], F32, tag="rope_t2")
    for dst, src in _SWAP:
        nc.vector.tensor_mul(t2[dst:dst + 32, :], psum[src:src + 32, :],
                             sin_sb[dst:dst + 32, :])
    nc.vector.tensor_add(out_ap, t1[:], t2[:])


def _build():
    nc = bacc.Bacc("TRN2", target_bir_lowering=False, debug=False, num_devices=8)
    ap = {}
    for name, shape, dt in [
        ("xTa", [KPAD, T], F32R), ("mTa", [KPAD, T], F32R),
        ("wq", [KPAD, 512], F32R), ("wk", [KPAD, 512], F32R),
        ("wv", [DIM, 520], F32R), ("wo", [512, DIM], F32R),
        ("qcos", [128, T], F32), ("qsin", [128, T], F32),
        ("kcos", [128, T], F32), ("ksin", [128, T], F32),
    ]:
        ap[name] = nc.dram_tensor(name, shape, dt, kind="ExternalInput").ap()
    out = nc.dram_tensor("out", [T, DIM], F32, kind="ExternalOutput").ap()

    with tile.TileContext(nc) as tc, ExitStack() as ctx:
        const = ctx.enter_context(tc.tile_pool(name="const", bufs=1))
        wpool = ctx.enter_context(tc.tile_pool(name="wpool", bufs=1))
        big = ctx.enter_context(tc.tile_pool(name="big", bufs=1))
        xs = ctx.enter_context(tc.tile_pool(name="xs", bufs=3))
        rp = ctx.enter_context(tc.tile_pool(name="rp", bufs=2))
        pp = ctx.enter_context(tc.tile_pool(name="pp", bufs=2))
        ost = ctx.enter_context(tc.tile_pool(name="ost", bufs=2))
        ps = ctx.enter_context(tc.tile_pool(name="ps", bufs=1, space="PSUM"))

        ones_row = const.tile([1, 64], F32)
        nc.any.memset(ones_row[:], 1.0)

        # resident weights
        wq_sb = [wpool.tile([128, 512], F32R, tag=f"wq{k}", name=f"wq{k}") for k in range(9)]
        wk_sb = [wpool.tile([128, 512], F32R, tag=f"wk{k}", name=f"wk{k}") for k in range(9)]
        wv_sb = [wpool.tile([128, 520], F32R, tag=f"wv{k}", name=f"wv{k}") for k in range(8)]
        wo_sb = [wpool.tile([128, DIM], F32R, tag=f"wo{k}", name=f"wo{k}") for k in range(4)]
        for k in range(9):
            nc.sync.dma_start(wq_sb[k][:], ap["wq"][128 * k:128 * (k + 1), :])
            nc.sync.dma_start(wk_sb[k][:], ap["wk"][128 * k:128 * (k + 1), :])
        for k in range(8):
            nc.sync.dma_start(wv_sb[k][:], ap["wv"][128 * k:128 * (k + 1), :])
        for k in range(4):
            nc.sync.dma_start(wo_sb[k][:], ap["wo"][128 * k:128 * (k + 1), :])

        # rope tables
        sc = {}
        for name in ("qcos", "qsin", "kcos", "ksin"):
            sc[name] = big.tile([128, T], F32, tag=name, name=name)
            nc.sync.dma_start(sc[name][:], ap[name][:])

        # ---- V projection: V_sb[mt] [128 tok, 520] bf16 (ones cols memset) ----
        v_sb = [big.tile([128, 520], BF16, tag=f"v{mt}", name=f"v{mt}") for mt in range(16)]
        for mt in range(16):
            pv = ps.tile([128, 520], F32, tag="sa")
            for k in range(8):
                blk = xs.tile([128, 128], F32R, tag="mblk")
                nc.sync.dma_start(
                    blk[:], ap["mTa"][128 * k:128 * (k + 1), 128 * mt:128 * (mt + 1)])
                nc.tensor.matmul(pv[:, 0:512], blk[:], wv_sb[k][:, 0:512],
                                 start=(k == 0), stop=(k == 7))
                nc.tensor.matmul(pv[:, 512:520], blk[:], wv_sb[k][:, 512:520],
                                 start=(k == 0), stop=(k == 7))
            nc.vector.tensor_copy(v_sb[mt][:], pv[:])
            nc.any.memset(v_sb[mt][:, 64::65], 1.0)

        # ---- Q / K projections (transposed layout) + RoPE ----
        qt_sb = [big.tile([128, T], BF16, tag=f"qt{m}", name=f"qt{m}") for m in range(4)]
        kt_sb = [big.tile([128, T], BF16, tag=f"kt{m}", name=f"kt{m}") for m in range(4)]
        for (src, wsb, dst, cosn, sinn) in (
                ("xTa", wq_sb, qt_sb, "qcos", "qsin"),
                ("mTa", wk_sb, kt_sb, "kcos", "ksin")):
            for n in range(4):
                cs = slice(512 * n, 512 * (n + 1))
                pr = [ps.tile([128, 512], F32, tag=t, name=f"pr_{t}")
                      for t in ("sa", "sb", "oa", "ob")]
                for k in range(9):
                    xk = xs.tile([128, 512], F32R, tag="xblk")
                    nc.sync.dma_start(xk[:], ap[src][128 * k:128 * (k + 1), cs])
                    for m in range(4):
                        nc.tensor.matmul(pr[m][:], wsb[k][:, 128 * m:128 * (m + 1)],
                                         xk[:], start=(k == 0), stop=(k == 8))
                for m in range(4):
                    _rope_evict(nc, rp, pr[m], sc[cosn][:, cs], sc[sinn][:, cs],
                                512, dst[m][:, cs])

        # ---- attention + output projection, per 1024-wide q chunk ----
        o_sb = [big.tile([128, 1024], F32R, tag=f"o{j}", name=f"o{j}") for j in range(4)]
        for qc in range(2):
            qs = slice(1024 * qc, 1024 * (qc + 1))
            for hp in range(4):
                ops = [ps.tile([65, 1024], F32, tag=t, name=f"ops_{t}") for t in ("oa", "ob")]
                for kt in range(16):
                    ks = slice(128 * kt, 128 * (kt + 1))
                    sps = [ps.tile([128, 1024], F32, tag=t, name=f"sps_{t}") for t in ("sa", "sb")]
                    for hi in range(2):
                        lo = 64 * hi
                        for nh in range(2):
                            qsl = slice(1024 * qc + 512 * nh, 1024 * qc + 512 * (nh + 1))
                            nc.tensor.matmul(
                                sps[hi][:, 512 * nh:512 * (nh + 1)],
                                kt_sb[hp][lo:lo + 64, ks],
                                qt_sb[hp][lo:lo + 64, qsl],
                                start=True, stop=True)
                    pt = [pp.tile([128, 1024], BF16, tag=t, name=f"pt_{t}") for t in ("pa", "pb")]
                    for hi in range(2):
                        nc.scalar.activation(pt[hi][:], sps[hi][:], EXP)
                    for hi in range(2):
                        vc = 130 * hp + 65 * hi
                        for nh in range(2):
                            nc.tensor.matmul(
                                ops[hi][:, 512 * nh:512 * (nh + 1)],
                                v_sb[kt][:, vc:vc + 65],
                                pt[hi][:, 512 * nh:512 * (nh + 1)],
                                start=(kt == 0), stop=(kt == 15))
                # normalize: o_sb[hp][64*hi:...] = ops[hi][0:64] * (1/denom)
                for hi in range(2):
                    rc = rp.tile([1, 1024], F32, tag="recip", bufs=1)
                    nc.vector.reciprocal(rc[:], ops[hi][64:65, :])
                    bc = ps.tile([64, 1024], F32, tag="sa")
                    for nh in range(2):
                        nc.tensor.matmul(bc[:, 512 * nh:512 * (nh + 1)], ones_row[:],
                                         rc[:, 512 * nh:512 * (nh + 1)],
                                         start=True, stop=True)
                    bcs = rp.tile([64, 1024], F32, tag="bcs", bufs=1)
                    nc.vector.tensor_copy(bcs[:], bc[:])
                    nc.vector.tensor_mul(o_sb[hp][64 * hi:64 * hi + 64, :],
                                         ops[hi][0:64, :], bcs[:])
            # output projection for this q chunk
            for m in range(8):
                tsl = slice(1024 * qc + 128 * m, 1024 * qc + 128 * (m + 1))
                for n2 in range(2):
                    po = ps.tile([128, 512], F32, tag=("sa" if n2 == 0 else "sb"))
                    for k4 in range(4):
                        nc.tensor.matmul(po[:], o_sb[k4][:, 128 * m:128 * (m + 1)],
                                         wo_sb[k4][:, 512 * n2:512 * (n2 + 1)],
                                         start=(k4 == 0), stop=(k4 == 3))
                    st = ost.tile([128, 512], F32, tag="st")
                    nc.vector.tensor_copy(st[:], po[:])
                    nc.sync.dma_start(out[tsl, 512 * n2:512 * (n2 + 1)], st[:])
    nc.compile()
    return nc


_NC = None
_LAST_RES = None


def kernel(x, memory, query_coords, memory_coords,
           Wq, bq, Wk, bk, Wv, bv, Wo, bo):
    global _NC
    x = np.asarray(x, np.float32)
    memory = np.asarray(memory, np.float32)
    args = (x, memory, np.asarray(query_coords, np.float32),
            np.asarray(memory_coords, np.float32),
            np.asarray(Wq, np.float32), np.asarray(bq, np.float32),
            np.asarray(Wk, np.float32), np.asarray(bk, np.float32),
            np.asarray(Wv, np.float32), np.asarray(bv, np.float32),
            np.asarray(Wo, np.float32), np.asarray(bo, np.float32))
    if _NC is None:
        _NC = _build()
    in_maps = [_prep_core(c, *args) for c in range(8)]
    res = run_bass_kernel_spmd(_NC, in_maps, list(range(8)))
    global _LAST_RES
    _LAST_RES = res
    Wv_, bv_, Wo_, bo_ = args[8], args[9], args[10], args[11]
    corr = (bv_ @ Wo_.T + bo_).astype(np.float32)
    out = np.empty((B, T, DIM), np.float32)
    for b in range(B):
        out[b] = res.results[2 * b]["out"] + res.results[2 * b + 1]["out"] + corr
    return out
